# revision 13
# baseline (speedup 1.0000x reference)
"""Trainium2 Bass kernel for nn_CvtNodeInitializer (gnn_message_passing).

Strategy (per the sharding hint: partition nodes, route edges by tail-node
owner, replicate the small projection weight):
  - Host: filter edges whose tail is a CVT node (only those contribute),
    sort by tail, and route each edge's feature rows (relation_tokens[e],
    node_tokens[e] -- the reference's "edge slot" quirk) to the core that
    owns the tail node. Nodes are partitioned contiguously: core c owns
    rows [c*25000, (c+1)*25000).
  - Device (SPMD, identical program on 8 cores): for each window of 128
    nodes, stream the window's (padded) transposed edge-feature block,
    project msg = X @ W.T with float32r matmuls, compute per-edge logits
    with a fused multiply-reduce, exponentiate, build a one-hot(seg)*q
    matrix, and segment-reduce agg = OH.T @ msg and den = OH.T @ 1 in one
    PSUM tile. Blend agg/den + shared_cvt into the node rows via a
    predicated copy and store the window contiguously.
  - Host: concatenate the per-core output slices.
"""

import sys

sys.path.insert(0, "/opt/trn_rl_repo")

import numpy as np

N_NODES = 200000
N_EDGES = 200000
HID = 256
NCORES = 8
P = 128

_PROGRAM_CACHE: dict = {}

# matmul input dtypes: "f32" (exact, 4 cyc/row) or "f32r" (TF32-like, full
# rate at >=256 moving cols). Tunable for the accuracy/speed tradeoff.
MSG_DT = "f32"
AGG_DT = "f32"


def _build_program(S: int, W: int, repeats: int = 1):
    """Build the per-core Bass program. S = padded edge slots per window
    (<= 128), W = windows per core. Identical across cores (SPMD)."""
    import concourse.bacc as bacc
    import concourse.mybir as mybir
    import concourse.tile as tile

    f32 = mybir.dt.float32
    i32 = mybir.dt.int32
    Alu = mybir.AluOpType
    Act = mybir.ActivationFunctionType
    mdt = mybir.dt.float32r if MSG_DT == "f32r" else f32
    adt = mybir.dt.float32r if AGG_DT == "f32r" else f32

    assert S <= 128

    nc = bacc.Bacc()
    xt = nc.declare_dram_parameter("xt", [W, P, 4 * S], mdt, isOutput=False)
    sc = nc.declare_dram_parameter("sc", [P, W], f32, isOutput=False)
    cv = nc.declare_dram_parameter("cv", [P, W], f32, isOutput=False)
    nod = nc.declare_dram_parameter("nod", [W, P, HID], f32, isOutput=False)
    wch = nc.declare_dram_parameter("wch", [P, 4 * HID], mdt, isOutput=False)
    att = nc.declare_dram_parameter("att", [P, HID], f32, isOutput=False)
    shr = nc.declare_dram_parameter("shr", [P, HID], f32, isOutput=False)
    out = nc.declare_dram_parameter("out", [W, P, HID], f32, isOutput=True)

    with tile.TileContext(nc) as tc:
        with (
            tc.tile_pool(name="const", bufs=1) as cpool,
            tc.tile_pool(name="x", bufs=3) as xpool,
            tc.tile_pool(name="msg", bufs=3) as mpool,
            tc.tile_pool(name="nodp", bufs=3) as npool,
            tc.tile_pool(name="small", bufs=4) as spool,
            tc.tile_pool(name="pmsg", bufs=2, space="PSUM") as pmpool,
            tc.tile_pool(name="pagg", bufs=2, space="PSUM") as papool,
        ):
            # --- one-time constants ---
            wtile = cpool.tile([P, 4 * HID], mdt)
            atile = cpool.tile([P, HID], f32)
            stile = cpool.tile([P, HID], f32)
            sctile = cpool.tile([P, W], f32)
            cvtile = cpool.tile([P, W], f32)
            io_i = cpool.tile([P, P], i32)
            io_f = cpool.tile([P, P], f32)
            ones = cpool.tile([P, 1], adt)
            nc.sync.dma_start(out=wtile[:], in_=wch[:])
            nc.sync.dma_start(out=atile[:], in_=att[:])
            nc.sync.dma_start(out=stile[:], in_=shr[:])
            nc.sync.dma_start(out=sctile[:], in_=sc[:])
            nc.sync.dma_start(out=cvtile[:], in_=cv[:])
            nc.gpsimd.iota(io_i[:], pattern=[[1, P]], base=0, channel_multiplier=0)
            nc.vector.tensor_copy(io_f[:], io_i[:])
            nc.gpsimd.memset(ones[:], 1.0)

            def body(w):
                xtile = xpool.tile([P, 4 * S], mdt, tag="xt")
                ntile = npool.tile([P, HID], f32, tag="nt")
                nc.sync.dma_start(out=xtile[:], in_=xt[w])
                nc.sync.dma_start(out=ntile[:], in_=nod[w])

                # msg = X @ W_msg.T   (PSUM [S, 256])
                pm = pmpool.tile([P, HID], f32, tag="pm")
                for c in range(4):
                    nc.tensor.matmul(
                        pm[:S, :],
                        lhsT=xtile[:, c * S:(c + 1) * S],
                        rhs=wtile[:, c * HID:(c + 1) * HID],
                        start=(c == 0),
                        stop=(c == 3),
                    )
                msg = mpool.tile([P, HID], adt, tag="msg")
                nc.scalar.activation(msg[:S, :], pm[:S, :], Act.Copy)

                # l = sum(msg * attn) per edge; q = exp(l)
                scr = mpool.tile([P, HID], f32, tag="scr")
                lq = spool.tile([P, 2], f32, tag="lq")
                nc.vector.tensor_tensor(
                    out=scr[:S, :], in0=msg[:S, :], in1=atile[:S, :],
                    op=Alu.mult,
                )
                nc.vector.reduce_sum(
                    out=lq[:S, 0:1], in_=scr[:S, :],
                    axis=mybir.AxisListType.X,
                )
                nc.scalar.activation(lq[:S, 1:2], lq[:S, 0:1], Act.Exp)

                # one-hot(seg_local) * q   [S edges x 128 segs]
                oh = spool.tile([P, P], adt, tag="oh")
                nc.vector.tensor_scalar(
                    out=oh[:S, :],
                    in0=io_f[:S, :],
                    scalar1=sctile[:S, w:w + 1],
                    scalar2=lq[:S, 1:2],
                    op0=Alu.is_equal,
                    op1=Alu.mult,
                )

                # agg = OH.T @ msg (cols 0:256), den = OH.T @ 1 (col 256)
                pa = papool.tile([P, HID + 16], f32, tag="pa")
                nc.tensor.matmul(
                    pa[:, 0:HID],
                    lhsT=oh[:S, :],
                    rhs=msg[:S, :],
                    start=True,
                    stop=True,
                )
                nc.tensor.matmul(
                    pa[:, HID:HID + 1],
                    lhsT=oh[:S, :],
                    rhs=ones[:S, :],
                    start=True,
                    stop=True,
                )

                # dsafe = den>0 ? den : 1 ; rec = 1/dsafe
                dn = spool.tile([P, 3], f32, tag="dn")
                nc.vector.tensor_copy(dn[:, 0:1], pa[:, HID:HID + 1])
                nc.vector.tensor_scalar(
                    out=dn[:, 1:2],
                    in0=dn[:, 0:1],
                    scalar1=0.0,
                    scalar2=dn[:, 0:1],
                    op0=Alu.is_le,
                    op1=Alu.add,
                )
                nc.vector.reciprocal(dn[:, 2:3], dn[:, 1:2])

                # comp = agg * rec + shared ; blend into node rows; store
                comp = mpool.tile([P, HID], f32, tag="comp")
                nc.scalar.activation(
                    comp[:], pa[:, 0:HID], Act.Copy, scale=dn[:, 2:3]
                )
                nc.vector.tensor_tensor(
                    out=comp[:], in0=comp[:], in1=stile[:], op=Alu.add
                )
                nc.vector.tensor_tensor(
                    out=comp[:], in0=comp[:], in1=ntile[:], op=Alu.subtract
                )
                nc.vector.tensor_scalar(
                    out=comp[:], in0=comp[:],
                    scalar1=cvtile[:, w:w + 1], scalar2=None, op0=Alu.mult,
                )
                nc.vector.tensor_tensor(
                    out=ntile[:], in0=ntile[:], in1=comp[:], op=Alu.add
                )
                nc.sync.dma_start(out=out[w], in_=ntile[:])

            if repeats == 1:
                for w in range(W):
                    body(w)
            else:
                with tc.For_i(0, repeats, 1) as _iv:
                    for w in range(W):
                        body(w)

    nc.compile()
    return nc


def _host_prep(node_tokens, relation_tokens, edge_index, node_is_cvt,
               shared_cvt, attn_vector, W_msg, n_cores=NCORES):
    """Index routing + per-core input construction. Returns (in_maps, S, W, NPC)."""
    n_nodes, hid = node_tokens.shape
    npc = n_nodes // n_cores
    assert npc * n_cores == n_nodes
    W = (npc + P - 1) // P  # windows per core
    npc_pad = W * P

    tails = np.asarray(edge_index[1], dtype=np.int64)
    cvt = np.asarray(node_is_cvt, dtype=bool)
    eids = np.nonzero(cvt[tails])[0]
    et = tails[eids]
    order = np.argsort(et, kind="stable")
    eids = eids[order]
    et = et[order]

    core = et // npc
    seg = et - core * npc
    win = (seg // P).astype(np.int64)
    gw = core * W + win
    cnt = np.bincount(gw, minlength=n_cores * W)
    starts = np.zeros(n_cores * W, dtype=np.int64)
    np.cumsum(cnt[:-1], out=starts[1:])
    rank = np.arange(len(gw)) - starts[gw]
    S = int(cnt.max())
    S = max(32, ((S + 15) // 16) * 16)
    assert S <= P, f"window edge count {cnt.max()} exceeds {P}"

    X = np.concatenate(
        [np.ascontiguousarray(relation_tokens)[eids],
         np.ascontiguousarray(node_tokens)[eids]], axis=1
    ).astype(np.float32)  # [ne, 2H]

    xt_all = np.zeros((n_cores, W, S, 2 * hid), np.float32)
    xt_all[core, win, rank] = X
    # [C,W,S,512] -> [C,W,128,4*S] with block[p, c*S+j] = X[j, c*128+p]
    xt_all = np.ascontiguousarray(
        xt_all.transpose(0, 1, 3, 2)
        .reshape(n_cores, W, 4, P, S)
        .transpose(0, 1, 3, 2, 4)
        .reshape(n_cores, W, P, 4 * S)
    )

    segl = np.full((n_cores, W, P), -1000.0, np.float32)
    segl[core, win, rank] = (seg % P).astype(np.float32)
    sc_all = np.ascontiguousarray(segl.transpose(0, 2, 1))
    cvtpad = np.zeros((n_cores, npc_pad), np.float32)
    cvtpad[:, :npc] = cvt.reshape(n_cores, npc).astype(np.float32)
    cv_all = np.ascontiguousarray(
        cvtpad.reshape(n_cores, W, P).transpose(0, 2, 1)
    )

    nod_all = np.zeros((n_cores, npc_pad, hid), np.float32)
    nod_all[:, :npc] = np.asarray(node_tokens, np.float32).reshape(
        n_cores, npc, hid
    )
    nod_all = nod_all.reshape(n_cores, W, P, hid)

    Wt = np.asarray(W_msg, np.float32).T  # [2H, H]
    wch = np.ascontiguousarray(
        Wt.reshape(4, P, hid).transpose(1, 0, 2).reshape(P, 4 * hid)
    )
    att = np.ascontiguousarray(
        np.broadcast_to(np.asarray(attn_vector, np.float32), (P, hid))
    )
    shr = np.ascontiguousarray(
        np.broadcast_to(np.asarray(shared_cvt, np.float32), (P, hid))
    )

    in_maps = [
        {
            "xt": xt_all[c],
            "sc": sc_all[c],
            "cv": cv_all[c],
            "nod": nod_all[c],
            "wch": wch,
            "att": att,
            "shr": shr,
        }
        for c in range(n_cores)
    ]
    return in_maps, S, W, npc


def kernel(**inputs) -> np.ndarray:
    from concourse import bass2jax

    node_tokens = np.asarray(inputs["node_tokens"], np.float32)
    in_maps, S, W, npc = _host_prep(
        node_tokens,
        inputs["relation_tokens"],
        inputs["edge_index"],
        inputs["node_is_cvt"],
        inputs["shared_cvt"],
        inputs["attn_vector"],
        inputs["W_msg"],
    )
    key = (S, W)
    nc = _PROGRAM_CACHE.get(key)
    if nc is None:
        nc = _build_program(S, W)
        _PROGRAM_CACHE[key] = nc
    results = bass2jax.run_bass_via_pjrt(nc, in_maps, n_cores=len(in_maps))
    hid = node_tokens.shape[1]
    return np.concatenate(
        [r["out"].reshape(-1, hid)[:npc] for r in results], axis=0
    )


# revision 16
# speedup vs baseline: 1.0668x; 1.0668x over previous
"""Trainium2 Bass kernel for nn_CvtNodeInitializer (gnn_message_passing).

Strategy (per the sharding hint: partition nodes, route edges by tail-node
owner, replicate the small projection weight):
  - Host: filter edges whose tail is a CVT node (only those contribute),
    sort by tail, and route each edge's feature rows (relation_tokens[e],
    node_tokens[e] -- the reference's "edge slot" quirk) to the core that
    owns the tail node. Nodes are partitioned contiguously: core c owns
    rows [c*25000, (c+1)*25000).
  - Device (SPMD, identical program on 8 cores): for each window of 128
    nodes, stream the window's (padded) transposed edge-feature block,
    project msg = X @ W.T with float32r matmuls, compute per-edge logits
    with a fused multiply-reduce, exponentiate, build a one-hot(seg)*q
    matrix, and segment-reduce agg = OH.T @ msg and den = OH.T @ 1 in one
    PSUM tile. Blend agg/den + shared_cvt into the node rows via a
    predicated copy and store the window contiguously.
  - Host: concatenate the per-core output slices.
"""

import sys

sys.path.insert(0, "/opt/trn_rl_repo")

import numpy as np

N_NODES = 200000
N_EDGES = 200000
HID = 256
NCORES = 8
P = 128

_PROGRAM_CACHE: dict = {}

# matmul input dtypes: "f32" (exact, 4 cyc/row) or "f32r" (TF32-like, full
# rate at >=256 moving cols). Tunable for the accuracy/speed tradeoff.
MSG_DT = "f32"
AGG_DT = "f32"


def _build_program(S: int, W: int, repeats: int = 1):
    """Build the per-core Bass program. S = padded edge slots per window
    (<= 128), W = windows per core. Identical across cores (SPMD)."""
    import concourse.bacc as bacc
    import concourse.mybir as mybir
    import concourse.tile as tile

    f32 = mybir.dt.float32
    i32 = mybir.dt.int32
    Alu = mybir.AluOpType
    Act = mybir.ActivationFunctionType
    mdt = mybir.dt.float32r if MSG_DT == "f32r" else f32
    adt = mybir.dt.float32r if AGG_DT == "f32r" else f32

    NT = (S + P - 1) // P  # slot tiles per window
    ST = S // NT
    assert ST * NT == S and ST <= P

    nc = bacc.Bacc()
    xt = nc.declare_dram_parameter("xt", [W, P, 4 * S], mdt, isOutput=False)
    sc = nc.declare_dram_parameter("sc", [P, W * NT], f32, isOutput=False)
    cv = nc.declare_dram_parameter("cv", [P, W], f32, isOutput=False)
    nod = nc.declare_dram_parameter("nod", [W, P, HID], f32, isOutput=False)
    wch = nc.declare_dram_parameter("wch", [P, 4 * HID], mdt, isOutput=False)
    att = nc.declare_dram_parameter("att", [P, HID], f32, isOutput=False)
    shr = nc.declare_dram_parameter("shr", [P, HID], f32, isOutput=False)
    out = nc.declare_dram_parameter("out", [W, P, HID], f32, isOutput=True)

    with tile.TileContext(nc) as tc:
        with (
            tc.tile_pool(name="const", bufs=1) as cpool,
            tc.tile_pool(name="x", bufs=3) as xpool,
            tc.tile_pool(name="msg", bufs=3) as mpool,
            tc.tile_pool(name="nodp", bufs=3) as npool,
            tc.tile_pool(name="small", bufs=4) as spool,
            tc.tile_pool(name="pmsg", bufs=2, space="PSUM") as pmpool,
            tc.tile_pool(name="pagg", bufs=2, space="PSUM") as papool,
            tc.tile_pool(name="pden", bufs=2, space="PSUM") as pdpool,
        ):
            # --- one-time constants ---
            wtile = cpool.tile([P, 4 * HID], mdt)
            atile = cpool.tile([P, HID], f32)
            stile = cpool.tile([P, HID], f32)
            sctile = cpool.tile([P, W * NT], f32)
            cvtile = cpool.tile([P, W], f32)
            io_i = cpool.tile([P, P], i32)
            io_f = cpool.tile([P, P], f32)
            ones = cpool.tile([P, 1], adt)
            nc.sync.dma_start(out=wtile[:], in_=wch[:])
            nc.sync.dma_start(out=atile[:], in_=att[:])
            nc.sync.dma_start(out=stile[:], in_=shr[:])
            nc.sync.dma_start(out=sctile[:], in_=sc[:])
            nc.sync.dma_start(out=cvtile[:], in_=cv[:])
            nc.gpsimd.iota(io_i[:], pattern=[[1, P]], base=0, channel_multiplier=0)
            nc.vector.tensor_copy(io_f[:], io_i[:])
            nc.gpsimd.memset(ones[:], 1.0)

            def body(w):
                pa = papool.tile([P, HID], f32, tag="pa")
                pd = pdpool.tile([P, 16], f32, tag="pd")
                ntile = npool.tile([P, HID], f32, tag="nt")
                nc.sync.dma_start(out=ntile[:], in_=nod[w])
                for t in range(NT):
                    xtile = xpool.tile([P, 4 * ST], mdt, tag="xt")
                    nc.sync.dma_start(
                        out=xtile[:],
                        in_=xt[w, :, t * 4 * ST:(t + 1) * 4 * ST])

                    # msg = X @ W_msg.T   (PSUM [ST, 256])
                    pm = pmpool.tile([P, HID], f32, tag="pm")
                    for c in range(4):
                        nc.tensor.matmul(
                            pm[:ST, :],
                            lhsT=xtile[:, c * ST:(c + 1) * ST],
                            rhs=wtile[:, c * HID:(c + 1) * HID],
                            start=(c == 0),
                            stop=(c == 3),
                        )
                    msg = mpool.tile([P, HID], adt, tag="msg")
                    nc.scalar.activation(msg[:ST, :], pm[:ST, :], Act.Copy)

                    # l = sum(msg * attn) per edge; q = exp(l)
                    scr = mpool.tile([P, HID], f32, tag="scr")
                    lq = spool.tile([P, 2], f32, tag="lq")
                    nc.vector.tensor_tensor(
                        out=scr[:ST, :], in0=msg[:ST, :], in1=atile[:ST, :],
                        op=Alu.mult,
                    )
                    nc.vector.reduce_sum(
                        out=lq[:ST, 0:1], in_=scr[:ST, :],
                        axis=mybir.AxisListType.X,
                    )
                    nc.scalar.activation(lq[:ST, 1:2], lq[:ST, 0:1], Act.Exp)

                    # one-hot(seg_local) * q   [ST edges x 128 segs]
                    oh = spool.tile([P, P], adt, tag="oh")
                    nc.vector.tensor_scalar(
                        out=oh[:ST, :],
                        in0=io_f[:ST, :],
                        scalar1=sctile[:ST, w * NT + t:w * NT + t + 1],
                        scalar2=lq[:ST, 1:2],
                        op0=Alu.is_equal,
                        op1=Alu.mult,
                    )

                    # agg += OH.T @ msg (cols 0:256), den += OH.T @ 1
                    nc.tensor.matmul(
                        pa[:, 0:HID],
                        lhsT=oh[:ST, :],
                        rhs=msg[:ST, :],
                        start=(t == 0),
                        stop=(t == NT - 1),
                    )
                    nc.tensor.matmul(
                        pd[:, 0:1],
                        lhsT=oh[:ST, :],
                        rhs=ones[:ST, :],
                        start=(t == 0),
                        stop=(t == NT - 1),
                    )

                # dsafe = den>0 ? den : 1 ; rec = 1/dsafe
                dn = spool.tile([P, 3], f32, tag="dn")
                nc.vector.tensor_copy(dn[:, 0:1], pd[:, 0:1])
                nc.vector.tensor_scalar(
                    out=dn[:, 1:2],
                    in0=dn[:, 0:1],
                    scalar1=0.0,
                    scalar2=dn[:, 0:1],
                    op0=Alu.is_le,
                    op1=Alu.add,
                )
                nc.vector.reciprocal(dn[:, 2:3], dn[:, 1:2])

                # comp = agg * rec + shared ; blend into node rows; store
                comp = mpool.tile([P, HID], f32, tag="comp")
                nc.scalar.activation(
                    comp[:], pa[:, 0:HID], Act.Copy, scale=dn[:, 2:3]
                )
                nc.vector.tensor_tensor(
                    out=comp[:], in0=comp[:], in1=stile[:], op=Alu.add
                )
                nc.vector.tensor_tensor(
                    out=comp[:], in0=comp[:], in1=ntile[:], op=Alu.subtract
                )
                nc.vector.tensor_scalar(
                    out=comp[:], in0=comp[:],
                    scalar1=cvtile[:, w:w + 1], scalar2=None, op0=Alu.mult,
                )
                nc.vector.tensor_tensor(
                    out=ntile[:], in0=ntile[:], in1=comp[:], op=Alu.add
                )
                nc.sync.dma_start(out=out[w], in_=ntile[:])

            if repeats == 1:
                for w in range(W):
                    body(w)
            else:
                with tc.For_i(0, repeats, 1) as _iv:
                    for w in range(W):
                        body(w)

    nc.compile()
    return nc


def _host_prep(node_tokens, relation_tokens, edge_index, node_is_cvt,
               shared_cvt, attn_vector, W_msg, n_cores=NCORES):
    """Index routing + per-core input construction. Returns (in_maps, S, W, NPC)."""
    n_nodes, hid = node_tokens.shape
    npc = n_nodes // n_cores
    assert npc * n_cores == n_nodes
    W = (npc + P - 1) // P  # windows per core
    npc_pad = W * P

    tails = np.asarray(edge_index[1], dtype=np.int64)
    cvt = np.asarray(node_is_cvt, dtype=bool)
    eids = np.nonzero(cvt[tails])[0]
    et = tails[eids]
    order = np.argsort(et, kind="stable")
    eids = eids[order]
    et = et[order]

    core = et // npc
    seg = et - core * npc
    win = (seg // P).astype(np.int64)
    gw = core * W + win
    cnt = np.bincount(gw, minlength=n_cores * W)
    starts = np.zeros(n_cores * W, dtype=np.int64)
    np.cumsum(cnt[:-1], out=starts[1:])
    rank = np.arange(len(gw)) - starts[gw]
    S = max(32, int(cnt.max()))
    NT = (S + P - 1) // P
    ST = ((S + NT - 1) // NT + 15) // 16 * 16
    S = ST * NT

    X = np.concatenate(
        [np.ascontiguousarray(relation_tokens)[eids],
         np.ascontiguousarray(node_tokens)[eids]], axis=1
    ).astype(np.float32)  # [ne, 2H]

    xt_all = np.zeros((n_cores, W, S, 2 * hid), np.float32)
    xt_all[core, win, rank] = X
    # [C,W,S,512] -> [C,W,128,NT*4*ST] with block[p, (t*4+c)*ST+j] =
    # X[slot t*ST+j, c*128+p]
    xt_all = np.ascontiguousarray(
        xt_all.transpose(0, 1, 3, 2)
        .reshape(n_cores, W, 4, P, NT, ST)
        .transpose(0, 1, 3, 4, 2, 5)
        .reshape(n_cores, W, P, 4 * S)
    )

    segl = np.full((n_cores, W, S), -1000.0, np.float32)
    segl[core, win, rank] = (seg % P).astype(np.float32)
    segf = np.full((n_cores, W * NT, P), -1000.0, np.float32)
    segf[:, :, :ST] = segl.reshape(n_cores, W * NT, ST)
    sc_all = np.ascontiguousarray(segf.transpose(0, 2, 1))
    cvtpad = np.zeros((n_cores, npc_pad), np.float32)
    cvtpad[:, :npc] = cvt.reshape(n_cores, npc).astype(np.float32)
    cv_all = np.ascontiguousarray(
        cvtpad.reshape(n_cores, W, P).transpose(0, 2, 1)
    )

    nod_all = np.zeros((n_cores, npc_pad, hid), np.float32)
    nod_all[:, :npc] = np.asarray(node_tokens, np.float32).reshape(
        n_cores, npc, hid
    )
    nod_all = nod_all.reshape(n_cores, W, P, hid)

    Wt = np.asarray(W_msg, np.float32).T  # [2H, H]
    wch = np.ascontiguousarray(
        Wt.reshape(4, P, hid).transpose(1, 0, 2).reshape(P, 4 * hid)
    )
    att = np.ascontiguousarray(
        np.broadcast_to(np.asarray(attn_vector, np.float32), (P, hid))
    )
    shr = np.ascontiguousarray(
        np.broadcast_to(np.asarray(shared_cvt, np.float32), (P, hid))
    )

    in_maps = [
        {
            "xt": xt_all[c],
            "sc": sc_all[c],
            "cv": cv_all[c],
            "nod": nod_all[c],
            "wch": wch,
            "att": att,
            "shr": shr,
        }
        for c in range(n_cores)
    ]
    return in_maps, S, W, npc


def kernel(**inputs) -> np.ndarray:
    from concourse import bass2jax

    node_tokens = np.asarray(inputs["node_tokens"], np.float32)
    in_maps, S, W, npc = _host_prep(
        node_tokens,
        inputs["relation_tokens"],
        inputs["edge_index"],
        inputs["node_is_cvt"],
        inputs["shared_cvt"],
        inputs["attn_vector"],
        inputs["W_msg"],
    )
    key = (S, W)
    nc = _PROGRAM_CACHE.get(key)
    if nc is None:
        nc = _build_program(S, W)
        _PROGRAM_CACHE[key] = nc
    results = bass2jax.run_bass_via_pjrt(nc, in_maps, n_cores=len(in_maps))
    hid = node_tokens.shape[1]
    return np.concatenate(
        [r["out"].reshape(-1, hid)[:npc] for r in results], axis=0
    )
